# revision 1
# baseline (speedup 1.0000x reference)
"""GroupLinear (block-diagonal 64x[64,64] linear) Trainium2 kernel.

Strategy:
  - Host: cast to fp16, transpose x ([8192, 4096] -> per-core [512, 8192]
    channel-major shards; group-parallel: core c owns groups [8c, 8c+8)),
    and pack the 8 diagonal weight blocks per core into 4 block-diagonal
    [128(in),128(out)] lhsT tiles (W^T layout, two groups per tile).
  - Device (per core): for each of the 4 channel-pair blocks, stream
    [128, 512] token chunks of x^T through the PE
    (out[128 out_ch, N tok] = lhsT.T @ rhs, fp16 in, fp32 PSUM accum),
    copy+downcast PSUM->SBUF, DMA out to y^T. All HBM traffic is
    contiguous; no on-device transposes.
  - Host: concatenate per-core y^T shards, transpose back, upcast to f32.

fp16 keeps scale-relative absmax error ~5e-4 on these inputs (f32 device
I/O is available via GL_DTYPE=f32 at ~2x the HBM traffic).
"""

import os
import sys

import numpy as np

for _p in ("/opt/trn_rl_repo", "/root/.axon_site/_ro/trn_rl_repo"):
    if os.path.isdir(_p) and _p not in sys.path:
        sys.path.insert(0, _p)

import concourse.bass as bass  # noqa: E402
import concourse.tile as tile  # noqa: E402
from concourse import bacc, mybir  # noqa: E402
from concourse.bass_utils import run_bass_kernel_spmd  # noqa: E402

N_CORES = 8
N_TOKENS = 8192
IN_CH = 4096
OUT_CH = 4096
GROUP_NUM = 64
SCALE = 64  # in_scale == out_scale == 64
GROUPS_PER_CORE = GROUP_NUM // N_CORES  # 8
CH_PER_CORE = IN_CH // N_CORES  # 512
PAIRS_PER_CORE = GROUPS_PER_CORE // 2  # 4 (two groups per 128-wide PE tile)
MM_N = 512  # one fp32 PSUM bank

LAST_RESULTS = None
_PROGRAMS = {}

_DTYPES = {
    "f16": (mybir.dt.float16, np.float16),
    "f32": (mybir.dt.float32, np.float32),
}


def _build_program(dtype_key: str, tok_chunk: int):
    dt, _ = _DTYPES[dtype_key]
    nc = bacc.Bacc(None, target_bir_lowering=False, debug=False)
    xt = nc.dram_tensor("xt", [CH_PER_CORE, N_TOKENS], dt, kind="ExternalInput")
    wt = nc.dram_tensor(
        "wt", [128, PAIRS_PER_CORE * 128], dt, kind="ExternalInput"
    )
    yt = nc.dram_tensor("yt", [CH_PER_CORE, N_TOKENS], dt, kind="ExternalOutput")
    xt_ap, wt_ap, yt_ap = xt.ap(), wt.ap(), yt.ap()

    # Chunk schedule per channel-pair block: small chunks at the very start
    # (fast pipeline ramp) and at the very end (short drain), big 2 MiB-class
    # chunks in the middle for DMA efficiency.
    chunk_lists = [[1024, 1024, 2048, 4096]]
    chunk_lists += [[4096, 4096]] * (PAIRS_PER_CORE - 2)
    chunk_lists += [[4096, 2048, 1024, 1024]]

    with tile.TileContext(nc) as tc:
        with (
            tc.tile_pool(name="wp", bufs=1) as wp,
            tc.tile_pool(name="xp", bufs=5) as xp,
            tc.tile_pool(name="yp", bufs=4) as yp,
            tc.tile_pool(name="ps", bufs=8, space="PSUM") as psp,
        ):
            w_sb = wp.tile([128, PAIRS_PER_CORE * 128], dt)
            # Single contiguous weight load, dispatched ahead of the x loads.
            nc.sync.dma_start(w_sb[:], wt_ap[:])
            cast_flip = 0
            for p in range(PAIRS_PER_CORE):
                t0 = 0
                for csz in chunk_lists[p]:
                    x_t = xp.tile([128, csz], dt, tag="x")
                    nc.sync.dma_start(
                        x_t[:],
                        xt_ap[p * 128 : (p + 1) * 128, t0 : t0 + csz],
                    )
                    y_t = yp.tile([128, csz], dt, tag="y")
                    for s in range(csz // MM_N):
                        ps = psp.tile([128, MM_N], mybir.dt.float32)
                        nc.tensor.matmul(
                            ps[:],
                            w_sb[:, p * 128 : (p + 1) * 128],
                            x_t[:, s * MM_N : (s + 1) * MM_N],
                            start=True,
                            stop=True,
                        )
                        # Alternate PSUM->SBUF downcasts across DVE and ACT
                        # so neither engine serializes the store path.
                        if cast_flip % 2 == 0:
                            nc.vector.tensor_copy(
                                y_t[:, s * MM_N : (s + 1) * MM_N], ps[:]
                            )
                        else:
                            nc.scalar.copy(
                                y_t[:, s * MM_N : (s + 1) * MM_N], ps[:]
                            )
                        cast_flip += 1
                    # Stores dispatch from the ACT HWDGE ring, parallel to
                    # the Sync ring carrying the loads.
                    nc.scalar.dma_start(
                        yt_ap[p * 128 : (p + 1) * 128, t0 : t0 + csz],
                        y_t[:],
                    )
                    t0 += csz
    nc.compile()
    return nc


def _chunk_schedule():
    """Per-pair chunk sizes: small at start (ramp) and end (drain)."""
    chunk_lists = [[1024, 1024, 2048, 4096]]
    chunk_lists += [[4096, 4096]] * (PAIRS_PER_CORE - 2)
    chunk_lists += [[4096, 2048, 1024, 1024]]
    chunks = []
    for p, lst in enumerate(chunk_lists):
        t0 = 0
        for csz in lst:
            chunks.append((p, t0, csz))
            t0 += csz
        assert t0 == N_TOKENS
    return chunks


def _build_program_raw(dtype_key: str, clear_sems: bool = True):
    """Hand-scheduled pipeline (no TileContext): avoids the Tile kernel-tail
    drain + all-engine barrier butterfly (~8.5 us).

    clear_sems=False only for CoreSim validation: the race detector cannot
    see that the end-of-program clear is ordered after every engine's last
    wait via the sem_done chain (scalar's terminal waits retire before
    sem_done increments, and every other engine's waits retire before the
    stores that sem_done transitively covers)."""
    dt, _ = _DTYPES[dtype_key]
    nc = bacc.Bacc(None, target_bir_lowering=False, debug=False)
    xt = nc.dram_tensor("xt", [CH_PER_CORE, N_TOKENS], dt, kind="ExternalInput")
    wt = nc.dram_tensor(
        "wt", [128, PAIRS_PER_CORE * 128], dt, kind="ExternalInput"
    )
    yt = nc.dram_tensor("yt", [CH_PER_CORE, N_TOKENS], dt, kind="ExternalOutput")
    xt_ap, wt_ap, yt_ap = xt.ap(), wt.ap(), yt.ap()

    chunks = _chunk_schedule()
    n_ch = len(chunks)
    X_SLOTS, Y_SLOTS, SLOT_W = 8, 6, 4096
    # global matmul index bookkeeping
    mm_of_chunk = [csz // MM_N for (_, _, csz) in chunks]
    mm_prefix = [0]
    for n in mm_of_chunk:
        mm_prefix.append(mm_prefix[-1] + n)
    n_mm = mm_prefix[-1]
    # cast engine per global mm index: even -> DVE, odd -> ACT
    cv_prefix = [0]  # DVE casts among mm [0, m)
    for m in range(n_mm):
        cv_prefix.append(cv_prefix[-1] + (1 if m % 2 == 0 else 0))

    with (
        nc.sbuf_tensor("xsb", [128, X_SLOTS * SLOT_W], dt) as xsb,
        nc.sbuf_tensor("ysb", [128, Y_SLOTS * SLOT_W], dt) as ysb,
        nc.sbuf_tensor("wsb", [128, PAIRS_PER_CORE * 128], dt) as wsb,
        nc.psum_tensor("pss", [128, 8 * MM_N], mybir.dt.float32) as pss,
        nc.Block() as block,
    ):
        # Per-DMA semaphores: concurrent DMAs interleave their 16 engine
        # increments, so a shared counting semaphore cannot attribute
        # completion to a specific transfer.
        sem_w = nc.alloc_semaphore("sem_w")
        sem_x = [nc.alloc_semaphore(f"sem_x{i}") for i in range(n_ch)]
        sem_st = [nc.alloc_semaphore(f"sem_st{i}") for i in range(n_ch)]
        sem_mm = nc.alloc_semaphore("sem_mm")
        sem_cv = nc.alloc_semaphore("sem_cv")
        sem_ca = nc.alloc_semaphore("sem_ca")
        sem_done = nc.alloc_semaphore("sem_done")
        all_sems = [sem_w, *sem_x, *sem_st, sem_mm, sem_cv, sem_ca, sem_done]
        sem_nums = sorted(s.num for s in all_sems)
        assert sem_nums == list(
            range(sem_nums[0], sem_nums[0] + len(sem_nums))
        ), "semaphore range not contiguous"

        def x_slot(i, csz):
            return xsb[:, (i % X_SLOTS) * SLOT_W :][:, :csz]

        def y_slot(i, csz):
            return ysb[:, (i % Y_SLOTS) * SLOT_W :][:, :csz]

        def bank(m):
            return pss[:, (m % 8) * MM_N : (m % 8 + 1) * MM_N]

        @block.sync
        def _(sync):
            sync.dma_start(wsb[:], wt_ap[:]).then_inc(sem_w, 16)
            for i, (p, t0, csz) in enumerate(chunks):
                if i >= X_SLOTS:
                    # slot reuse: all matmuls of chunk i-X_SLOTS retired
                    sync.wait_ge(sem_mm, mm_prefix[i - X_SLOTS + 1])
                sync.dma_start(
                    x_slot(i, csz),
                    xt_ap[p * 128 : (p + 1) * 128, t0 : t0 + csz],
                ).then_inc(sem_x[i], 16)

        @block.tensor
        def _(tensor):
            tensor.wait_ge(sem_w, 16)
            m = 0
            for i, (p, t0, csz) in enumerate(chunks):
                tensor.wait_ge(sem_x[i], 16)
                for s in range(csz // MM_N):
                    if m >= 8:
                        j = m - 8  # bank reuse: cast j must have retired
                        if j % 2 == 0:
                            tensor.wait_ge(sem_cv, j // 2 + 1)
                        else:
                            tensor.wait_ge(sem_ca, j // 2 + 1)
                    tensor.matmul(
                        bank(m),
                        wsb[:, p * 128 : (p + 1) * 128],
                        x_slot(i, csz)[:, s * MM_N : (s + 1) * MM_N],
                        start=True,
                        stop=True,
                    ).then_inc(sem_mm)
                    m += 1

        @block.vector
        def _(vector):
            m = 0
            for i, (p, t0, csz) in enumerate(chunks):
                first_in_chunk = True
                for s in range(csz // MM_N):
                    if m % 2 == 0:
                        if first_in_chunk and i >= Y_SLOTS:
                            vector.wait_ge(sem_st[i - Y_SLOTS], 16)
                        first_in_chunk = False
                        vector.wait_ge(sem_mm, m + 1)
                        vector.tensor_copy(
                            y_slot(i, csz)[:, s * MM_N : (s + 1) * MM_N],
                            bank(m),
                        ).then_inc(sem_cv)
                    m += 1

        @block.scalar
        def _(scalar):
            m = 0
            for i, (p, t0, csz) in enumerate(chunks):
                first_in_chunk = True
                for s in range(csz // MM_N):
                    if m % 2 == 1:
                        if first_in_chunk and i >= Y_SLOTS:
                            scalar.wait_ge(sem_st[i - Y_SLOTS], 16)
                        first_in_chunk = False
                        scalar.wait_ge(sem_mm, m + 1)
                        scalar.copy(
                            y_slot(i, csz)[:, s * MM_N : (s + 1) * MM_N],
                            bank(m),
                        ).then_inc(sem_ca)
                    m += 1
                # store chunk i: the DMA reads the y slot asynchronously, so
                # wait on BOTH engines' cast-completion counts.
                scalar.wait_ge(sem_cv, cv_prefix[mm_prefix[i + 1]])
                scalar.wait_ge(sem_ca, mm_prefix[i + 1] - cv_prefix[mm_prefix[i + 1]])
                scalar.dma_start(
                    yt_ap[p * 128 : (p + 1) * 128, t0 : t0 + csz],
                    y_slot(i, csz),
                ).then_inc(sem_st[i], 16)
            for i in range(n_ch):
                scalar.wait_ge(sem_st[i], 16)
            scalar.nop().then_inc(sem_done)

        if clear_sems:

            @block.gpsimd
            def _(gpsimd):
                # Reset all semaphores after everything retired so the NEFF
                # can be re-executed (PJRT may run the loaded executable
                # again). sem_done >= 1 implies every other wait in the
                # program retired; the terminal-value waits below all pass
                # instantly and exist so the clear happens-after every
                # update.
                gpsimd.wait_ge(sem_done, 1)
                rng = range(sem_nums[0], sem_nums[-1] + 1)
                gpsimd.dma_reset(rng)
                gpsimd.sem_clear(rng)

    nc.compile()
    return nc


def kernel(x: np.ndarray, weight: np.ndarray) -> np.ndarray:
    global LAST_RESULTS
    x = np.asarray(x)
    weight = np.asarray(weight, dtype=np.float32)
    assert x.shape == (N_TOKENS, IN_CH), x.shape
    assert weight.shape == (OUT_CH, IN_CH), weight.shape

    dtype_key = os.environ.get("GL_DTYPE", "f16")
    impl = os.environ.get("GL_IMPL", "raw")
    tok_chunk = int(os.environ.get("GL_TOK_CHUNK", "4096"))
    _, npdt = _DTYPES[dtype_key]

    key = (dtype_key, impl, tok_chunk)
    if key not in _PROGRAMS:
        if impl == "raw":
            _PROGRAMS[key] = _build_program_raw(dtype_key)
        else:
            _PROGRAMS[key] = _build_program(dtype_key, tok_chunk)
    nc = _PROGRAMS[key]

    # Diagonal blocks: blocks[g] = weight[g*64:(g+1)*64, g*64:(g+1)*64]
    wb = weight.reshape(GROUP_NUM, SCALE, GROUP_NUM, SCALE)
    idx = np.arange(GROUP_NUM)
    blocks = wb[idx, :, idx, :]  # [64, out 64, in 64]

    x_c = np.asarray(x, dtype=npdt)
    in_maps = []
    for c in range(N_CORES):
        xt_c = np.ascontiguousarray(
            x_c[:, c * CH_PER_CORE : (c + 1) * CH_PER_CORE].T
        )
        wt_c = np.zeros((128, PAIRS_PER_CORE * 128), npdt)
        for p in range(PAIRS_PER_CORE):
            g0 = c * GROUPS_PER_CORE + 2 * p
            base = p * 128
            wt_c[0:SCALE, base : base + SCALE] = blocks[g0].T.astype(
                npdt
            )  # [in, out]
            wt_c[SCALE:128, base + SCALE : base + 128] = blocks[g0 + 1].T.astype(
                npdt
            )
        in_maps.append({"xt": xt_c, "wt": wt_c})

    trace = os.environ.get("GL_TRACE") == "1"
    res = run_bass_kernel_spmd(
        nc, in_maps, core_ids=list(range(N_CORES)), trace=trace
    )
    LAST_RESULTS = res

    yt_full = np.concatenate(
        [r["yt"] for r in res.results], axis=0
    )  # [4096, 8192]
    return np.ascontiguousarray(yt_full.T.astype(np.float32))


if __name__ == "__main__":
    rng = np.random.default_rng(0)
    x = rng.standard_normal((N_TOKENS, IN_CH), dtype=np.float32)
    w = rng.standard_normal((OUT_CH, IN_CH), dtype=np.float32) / 64.0
    y = kernel(x, w)
    print("out", y.shape, y.dtype)

